# revision 15
# baseline (speedup 1.0000x reference)
"""Grok1 MoE (unfused) Trainium2 Bass kernel — sparse expert-parallel.

Top-2-of-8 routing means only ~T/4 of the 2048 tokens are live per
expert.  The router (a 2048x1024x8 matmul + softmax + top-2, ~34 MFLOP)
runs on host as part of the sharding step; each of the 8 NeuronCores
owns one expert and receives only that expert's routed tokens, gathered
and padded to a fixed capacity C:

  core e computes  outT = w2 @ (gelu(w1 @ x1) * (w3 @ x3))  [H, C]

with x1 = x[tids_e].T and x3 = (combine_e * x[tids_e]).T — the combine
weight is folded into the w3 operand (u is linear in x), so no
post-scale pass is needed.  Host scatter-adds the 8 partial outputs
back to the full [T, H] result.

Device schedule notes:
  - all inputs are pre-tiled on host into the exact SBUF partition
    layout, so every DMA reads multi-KB contiguous runs per partition
    (cheap descriptors, fast queue dispatch);
  - w1/w3 are fused into one tensor (one DMA per f-chunk); x streams in
    h-chunk halves so the first matmul can issue right after the NEFF
    preamble;
  - each 128x128 weight tile stays stationary for both token blocks
    (b0/b1 interleaved into separate PSUM banks);
  - fp16 matmuls accumulate in fp32 PSUM; partial outputs evict as fp16.
"""

import numpy as np

import concourse.bass as bass
import concourse.mybir as mybir
import concourse.tile as tile
from concourse import bacc
from concourse.bass import ts
from concourse.bass_utils import run_bass_kernel_spmd

T, H, F, E = 2048, 1024, 4096, 8
NCORES = 8
TOP_K = 2
HC = H // 128   # 8 h-chunks
FC = F // 128   # 32 f-chunks

f32 = mybir.dt.float32
f16 = mybir.dt.float16

_CACHE = {}


def build_nc(C, BLK):
    """Per-expert MLP over C tokens (2 blocks of BLK)."""
    nc = bacc.Bacc(
        "TRN2",
        target_bir_lowering=False,
        debug=False,
        num_devices=NCORES,
    )

    # host-pre-tiled layouts (partition-contiguous):
    #   xd      [128, HC*2*C]    : row p, col (c*2+j)*C+t = x{1,3}.T[c*128+p, t]
    #   w13d    [FC*128, 2*HC*128]: row f*128+p, col j*HC*128+c*128+m
    #                               = w{1,3}.T[c*128+p, f*128+m]
    #   w2d     [HC*128, FC*128] : row h*128+p, col f*128+m
    #                               = w2.T[f*128+p, h*128+m]
    xd = nc.dram_tensor("xd", [128, HC * 2 * C], f16, kind="ExternalInput")
    w13d = nc.dram_tensor("w13d", [FC * 128, 2 * HC * 128], f16, kind="ExternalInput")
    w2d = nc.dram_tensor("w2d", [HC * 128, FC * 128], f16, kind="ExternalInput")
    out = nc.dram_tensor("out", [H, C], f16, kind="ExternalOutput")

    AF = mybir.ActivationFunctionType

    with tile.TileContext(nc) as tc:
        with (
            tc.tile_pool(name="singles", bufs=1) as singles,
            tc.tile_pool(name="wpool", bufs=3) as wpool,
            tc.tile_pool(name="w2pool", bufs=3) as w2pool,
            tc.tile_pool(name="evict", bufs=2) as evict,
            tc.tile_pool(name="psum_gu", bufs=1, space="PSUM") as psum_gu,
            tc.tile_pool(name="psum_o", bufs=2, space="PSUM") as psum_o,
        ):
            def load_wf(f):
                # two half-DMAs: the g-matmuls only depend on the w1 half
                w1f = wpool.tile([128, HC, 128], f16, tag="w1f", name=f"w1f_{f}")
                nc.sync.dma_start(
                    out=w1f,
                    in_=w13d.ap()[ts(f, 128), 0 : HC * 128].rearrange(
                        "p (c m) -> p c m", c=HC
                    ),
                )
                w3f = wpool.tile([128, HC, 128], f16, tag="w3f", name=f"w3f_{f}")
                nc.sync.dma_start(
                    out=w3f,
                    in_=w13d.ap()[ts(f, 128), HC * 128 : 2 * HC * 128].rearrange(
                        "p (c m) -> p c m", c=HC
                    ),
                )
                return w1f, w3f

            # weights dispatch on the sync queue; x streams concurrently on
            # the scalar queue in graduated pieces (1,1,2,4 h-chunks) so the
            # first g-matmuls start right after the NEFF preamble
            wf_next = load_wf(0)
            x_sb = singles.tile([128, HC, 2, C], f16)
            lo = 0
            for npc in (1, 1, 2, 4):
                nc.scalar.dma_start(
                    out=x_sb[:, lo : lo + npc, :, :],
                    in_=xd.ap()[:, lo * 2 * C : (lo + npc) * 2 * C].rearrange(
                        "p (c j t) -> p c j t", c=npc, j=2
                    ),
                )
                lo += npc
            x1_sb = x_sb[:, :, 0, :]
            x3_sb = x_sb[:, :, 1, :]
            gus = singles.tile([128, FC, C], f16)

            # ---- phase W: g/u matmuls + gelu + mul -> gus (fp16) ----
            for f in range(FC):
                w1f, w3f = wf_next
                wf_next = load_wf(f + 1) if f + 1 < FC else None
                gb = [
                    psum_gu.tile([128, BLK], f32, tag=f"g{b}", name=f"g{b}_{f}")
                    for b in range(2)
                ]
                ub = [
                    psum_gu.tile([128, BLK], f32, tag=f"u{b}", name=f"u{b}_{f}")
                    for b in range(2)
                ]
                for h in range(HC):
                    for b in range(2):
                        nc.tensor.matmul(
                            gb[b],
                            lhsT=w1f[:, h, :],
                            rhs=x1_sb[:, h, ts(b, BLK)],
                            start=(h == 0),
                            stop=(h == HC - 1),
                        )
                gs = []
                for b in range(2):
                    g_sb = evict.tile([128, BLK], f32, tag=f"gs{b}", name=f"gs{b}_{f}")
                    nc.scalar.activation(g_sb, gb[b], AF.Gelu)
                    gs.append(g_sb)
                for h in range(HC):
                    for b in range(2):
                        nc.tensor.matmul(
                            ub[b],
                            lhsT=w3f[:, h, :],
                            rhs=x3_sb[:, h, ts(b, BLK)],
                            start=(h == 0),
                            stop=(h == HC - 1),
                        )
                for b in range(2):
                    nc.vector.tensor_mul(gus[:, f, ts(b, BLK)], gs[b], ub[b])

            # ---- phase M2: out[h,:] = w2 @ gus ----
            for h in range(HC):
                halves = []
                for hv in range(2):
                    w2h = w2pool.tile(
                        [128, FC // 2, 128], f16, tag=f"w2h{hv}", name=f"w2h{hv}_{h}"
                    )
                    nc.sync.dma_start(
                        out=w2h,
                        in_=w2d.ap()[
                            ts(h, 128), hv * (FC // 2) * 128 : (hv + 1) * (FC // 2) * 128
                        ].rearrange("p (c m) -> p c m", c=FC // 2),
                    )
                    halves.append(w2h)
                # last h: 4 narrow blocks so the final evictions overlap the
                # remaining matmul chains instead of trailing the kernel
                nblk, blk = (2, BLK) if h < HC - 1 else (4, BLK // 2)
                ob = [
                    psum_o.tile([128, blk], f32, tag=f"o{b % 2}", name=f"o{b}_{h}")
                    for b in range(nblk)
                ]
                for f in range(FC):
                    for b in range(nblk):
                        nc.tensor.matmul(
                            ob[b],
                            lhsT=halves[f // (FC // 2)][:, f % (FC // 2), :],
                            rhs=gus[:, f, ts(b, blk)],
                            start=(f == 0),
                            stop=(f == FC - 1),
                        )
                for b in range(nblk):
                    o_sb = evict.tile(
                        [128, blk], f16, tag=f"osb{b % 2}", name=f"osb{b}_{h}"
                    )
                    nc.scalar.copy(o_sb, ob[b])
                    nc.scalar.dma_start(out=out.ap()[ts(h, 128), ts(b, blk)], in_=o_sb)
    nc.finalize()
    return nc


def _route(hidden_states, gate_w):
    """fp32 router identical to the reference: softcapped logits ->
    softmax -> top-2 -> combine weights [T, E]."""
    logits = hidden_states @ gate_w.T
    logits = (30.0 * np.tanh(logits / 30.0)).astype(np.float32)
    lmax = logits.max(axis=-1, keepdims=True)
    p = np.exp(logits - lmax)
    probs = p / p.sum(axis=-1, keepdims=True)
    idx = np.argsort(-probs, axis=-1, kind="stable")[:, :TOP_K]
    vals = np.take_along_axis(probs, idx, axis=-1)
    combine = np.zeros((T, E), dtype=np.float32)
    np.put_along_axis(combine, idx, vals, axis=-1)
    return combine


def _get_nc(C):
    key = ("nc", C)
    if key not in _CACHE:
        BLK = C // 2
        assert BLK * 2 == C and BLK % 16 == 0 and BLK <= 512, (C, BLK)
        _CACHE[key] = build_nc(C, BLK)
    return _CACHE[key]


def _tile_x(x1T, x3T):
    """two [H, C] -> [128, HC*2*C]: row p, col (c*2+j)*C+t = x{1,3}T[c*128+p, t]."""
    HH, C = x1T.shape
    both = np.stack([x1T, x3T], axis=0)                  # [2, H, C] -> j, (c p), t
    return np.ascontiguousarray(
        both.reshape(2, HC, 128, C)                      # j c p t
        .transpose(2, 1, 0, 3)                           # p c j t
        .reshape(128, HC * 2 * C)
    )


def kernel(hidden_states, gate_w, w1, w2, w3, trace=False):
    hidden_states = np.asarray(hidden_states, dtype=np.float32)
    gate_w = np.asarray(gate_w, dtype=np.float32)
    w1 = np.asarray(w1, dtype=np.float32)
    w2 = np.asarray(w2, dtype=np.float32)
    w3 = np.asarray(w3, dtype=np.float32)

    combine = _route(hidden_states, gate_w)
    tids = [np.nonzero(combine[:, e])[0] for e in range(E)]
    max_n = max(len(t) for t in tids)
    # fixed capacity; bump in 32-token steps if an input routes more
    C = max(544, -(-max_n // 32) * 32)
    nc = _get_nc(C)

    in_maps = []
    for e in range(NCORES):
        n = len(tids[e])
        xg = hidden_states[tids[e]]                       # [n, H]
        ce = combine[tids[e], e][:, None]                 # [n, 1]
        x1p = np.zeros((C, H), dtype=np.float16)
        x1p[:n] = xg
        x3p = np.zeros((C, H), dtype=np.float16)
        x3p[:n] = xg * ce

        # w13d[f*128+p, j*HC*128 + c*128 + m] = w{1,3}[e][f*128+m, c*128+p]
        w13 = np.stack(
            [w1[e].astype(np.float16), w3[e].astype(np.float16)], axis=0
        )  # [2, F, H]
        w13d = np.ascontiguousarray(
            w13.reshape(2, FC, 128, HC, 128)     # j, f, m, c, p
            .transpose(1, 4, 0, 3, 2)            # f, p, j, c, m
            .reshape(FC * 128, 2 * HC * 128)
        )
        # w2d[h*128+p, f*128+m] = w2[e].T[f*128+p, h*128+m] = w2[e][h*128+m, f*128+p]
        w2d = np.ascontiguousarray(
            w2[e].astype(np.float16)
            .reshape(HC, 128, FC, 128)           # h, m, f, p
            .transpose(0, 3, 2, 1)               # h, p, f, m
            .reshape(HC * 128, FC * 128)
        )
        in_maps.append(
            {
                "xd": _tile_x(x1p.T, x3p.T),
                "w13d": w13d,
                "w2d": w2d,
            }
        )

    res = run_bass_kernel_spmd(nc, in_maps, core_ids=list(range(NCORES)), trace=trace)
    out = np.zeros((T, H), dtype=np.float32)
    for e, r in enumerate(res.results):
        n = len(tids[e])
        out[tids[e]] += r["out"][:, :n].T.astype(np.float32)
    _CACHE["last_results"] = res
    return out


if __name__ == "__main__":
    rng = np.random.default_rng(0)
    hs = rng.standard_normal((T, H), dtype=np.float32)
    gw = (rng.standard_normal((E, H)) * 0.02).astype(np.float32)
    w1 = (rng.standard_normal((E, F, H)) * 0.02).astype(np.float32)
    w2 = (rng.standard_normal((E, H, F)) * 0.02).astype(np.float32)
    w3 = (rng.standard_normal((E, F, H)) * 0.02).astype(np.float32)
    out = kernel(hs, gw, w1, w2, w3)
    print("out", out.shape, out.dtype, np.abs(out).max())


# revision 17
# speedup vs baseline: 1.0276x; 1.0276x over previous
"""Grok1 MoE (unfused) Trainium2 Bass kernel — sparse expert-parallel.

Top-2-of-8 routing means only ~T/4 of the 2048 tokens are live per
expert.  The router (a 2048x1024x8 matmul + softmax + top-2, ~34 MFLOP)
runs on host as part of the sharding step; each of the 8 NeuronCores
owns one expert and receives only that expert's routed tokens, gathered
and padded to a fixed capacity C:

  core e computes  outT = w2 @ (gelu(w1 @ x1) * (w3 @ x3))  [H, C]

with x1 = x[tids_e].T and x3 = (combine_e * x[tids_e]).T — the combine
weight is folded into the w3 operand (u is linear in x), so no
post-scale pass is needed.  Host scatter-adds the 8 partial outputs
back to the full [T, H] result.

Device schedule notes:
  - all inputs are pre-tiled on host into the exact SBUF partition
    layout, so every DMA reads multi-KB contiguous runs per partition
    (cheap descriptors, fast queue dispatch);
  - w1/w3 are fused into one tensor (one DMA per f-chunk); x streams in
    h-chunk halves so the first matmul can issue right after the NEFF
    preamble;
  - each 128x128 weight tile stays stationary for both token blocks
    (b0/b1 interleaved into separate PSUM banks);
  - fp16 matmuls accumulate in fp32 PSUM; partial outputs evict as fp16.
"""

import numpy as np

import concourse.bass as bass
import concourse.mybir as mybir
import concourse.tile as tile
from concourse import bacc
from concourse.bass import ts
from concourse.bass_utils import run_bass_kernel_spmd

T, H, F, E = 2048, 1024, 4096, 8
NCORES = 8
TOP_K = 2
HC = H // 128   # 8 h-chunks
FC = F // 128   # 32 f-chunks

f32 = mybir.dt.float32
f16 = mybir.dt.float16

_CACHE = {}


def build_nc(C, BLK):
    """Per-expert MLP over C tokens (2 blocks of BLK)."""
    nc = bacc.Bacc(
        "TRN2",
        target_bir_lowering=False,
        debug=False,
        num_devices=NCORES,
    )

    # host-pre-tiled layouts (partition-contiguous):
    #   xd      [128, HC*2*C]    : row p, col (c*2+j)*C+t = x{1,3}.T[c*128+p, t]
    #   w13d    [FC*128, 2*HC*128]: row f*128+p, col j*HC*128+c*128+m
    #                               = w{1,3}.T[c*128+p, f*128+m]
    #   w2d     [HC*128, FC*128] : row h*128+p, col f*128+m
    #                               = w2.T[f*128+p, h*128+m]
    xd = nc.dram_tensor("xd", [128, HC * 2 * C], f16, kind="ExternalInput")
    w13d = nc.dram_tensor("w13d", [FC * 128, 2 * HC * 128], f16, kind="ExternalInput")
    w2d = nc.dram_tensor("w2d", [HC * 128, FC * 128], f16, kind="ExternalInput")
    out = nc.dram_tensor("out", [H, C], f16, kind="ExternalOutput")

    AF = mybir.ActivationFunctionType

    with tile.TileContext(nc) as tc:
        with (
            tc.tile_pool(name="singles", bufs=1) as singles,
            tc.tile_pool(name="wpool", bufs=3) as wpool,
            tc.tile_pool(name="w2pool", bufs=3) as w2pool,
            tc.tile_pool(name="evict", bufs=2) as evict,
            tc.tile_pool(name="psum_gu", bufs=1, space="PSUM") as psum_gu,
            tc.tile_pool(name="psum_o", bufs=2, space="PSUM") as psum_o,
        ):
            def load_wf(f):
                # two half-DMAs: the g-matmuls only depend on the w1 half
                w1f = wpool.tile([128, HC, 128], f16, tag="w1f", name=f"w1f_{f}")
                nc.sync.dma_start(
                    out=w1f,
                    in_=w13d.ap()[ts(f, 128), 0 : HC * 128].rearrange(
                        "p (c m) -> p c m", c=HC
                    ),
                )
                w3f = wpool.tile([128, HC, 128], f16, tag="w3f", name=f"w3f_{f}")
                nc.sync.dma_start(
                    out=w3f,
                    in_=w13d.ap()[ts(f, 128), HC * 128 : 2 * HC * 128].rearrange(
                        "p (c m) -> p c m", c=HC
                    ),
                )
                return w1f, w3f

            # startup queue order: w1 half of f=0, first x quarter, w3 half,
            # remaining x quarters — so the first g-matmuls start right
            # after the NEFF preamble
            x_sb = singles.tile([128, HC, 2, C], f16)

            def load_xq(lo):
                nc.sync.dma_start(
                    out=x_sb[:, lo : lo + 2, :, :],
                    in_=xd.ap()[:, lo * 2 * C : (lo + 2) * 2 * C].rearrange(
                        "p (c j t) -> p c j t", c=2, j=2
                    ),
                )

            w1f0 = wpool.tile([128, HC, 128], f16, tag="w1f", name="w1f_0")
            nc.sync.dma_start(
                out=w1f0,
                in_=w13d.ap()[ts(0, 128), 0 : HC * 128].rearrange(
                    "p (c m) -> p c m", c=HC
                ),
            )
            load_xq(0)
            w3f0 = wpool.tile([128, HC, 128], f16, tag="w3f", name="w3f_0")
            nc.sync.dma_start(
                out=w3f0,
                in_=w13d.ap()[ts(0, 128), HC * 128 : 2 * HC * 128].rearrange(
                    "p (c m) -> p c m", c=HC
                ),
            )
            for lo in range(2, HC, 2):
                load_xq(lo)
            wf_next = (w1f0, w3f0)
            x1_sb = x_sb[:, :, 0, :]
            x3_sb = x_sb[:, :, 1, :]
            gus = singles.tile([128, FC, C], f16)

            # ---- phase W: g/u matmuls + gelu + mul -> gus (fp16) ----
            for f in range(FC):
                w1f, w3f = wf_next
                wf_next = load_wf(f + 1) if f + 1 < FC else None
                gb = [
                    psum_gu.tile([128, BLK], f32, tag=f"g{b}", name=f"g{b}_{f}")
                    for b in range(2)
                ]
                ub = [
                    psum_gu.tile([128, BLK], f32, tag=f"u{b}", name=f"u{b}_{f}")
                    for b in range(2)
                ]
                for h in range(HC):
                    for b in range(2):
                        nc.tensor.matmul(
                            gb[b],
                            lhsT=w1f[:, h, :],
                            rhs=x1_sb[:, h, ts(b, BLK)],
                            start=(h == 0),
                            stop=(h == HC - 1),
                        )
                gs = []
                for b in range(2):
                    g_sb = evict.tile([128, BLK], f32, tag=f"gs{b}", name=f"gs{b}_{f}")
                    nc.scalar.activation(g_sb, gb[b], AF.Gelu)
                    gs.append(g_sb)
                for h in range(HC):
                    for b in range(2):
                        nc.tensor.matmul(
                            ub[b],
                            lhsT=w3f[:, h, :],
                            rhs=x3_sb[:, h, ts(b, BLK)],
                            start=(h == 0),
                            stop=(h == HC - 1),
                        )
                for b in range(2):
                    nc.vector.tensor_mul(gus[:, f, ts(b, BLK)], gs[b], ub[b])

            # ---- phase M2: out[h,:] = w2 @ gus ----
            for h in range(HC):
                halves = []
                for hv in range(2):
                    w2h = w2pool.tile(
                        [128, FC // 2, 128], f16, tag=f"w2h{hv}", name=f"w2h{hv}_{h}"
                    )
                    nc.sync.dma_start(
                        out=w2h,
                        in_=w2d.ap()[
                            ts(h, 128), hv * (FC // 2) * 128 : (hv + 1) * (FC // 2) * 128
                        ].rearrange("p (c m) -> p c m", c=FC // 2),
                    )
                    halves.append(w2h)
                ob = [
                    psum_o.tile([128, BLK], f32, tag=f"o{b}", name=f"o{b}_{h}")
                    for b in range(2)
                ]
                for f in range(FC):
                    for b in range(2):
                        nc.tensor.matmul(
                            ob[b],
                            lhsT=halves[f // (FC // 2)][:, f % (FC // 2), :],
                            rhs=gus[:, f, ts(b, BLK)],
                            start=(f == 0),
                            stop=(f == FC - 1),
                        )
                for b in range(2):
                    o_sb = evict.tile(
                        [128, BLK], f16, tag=f"osb{b}", name=f"osb{b}_{h}"
                    )
                    nc.scalar.copy(o_sb, ob[b])
                    nc.sync.dma_start(out=out.ap()[ts(h, 128), ts(b, BLK)], in_=o_sb)
    nc.finalize()
    return nc


def _route(hidden_states, gate_w):
    """fp32 router identical to the reference: softcapped logits ->
    softmax -> top-2 -> combine weights [T, E]."""
    logits = hidden_states @ gate_w.T
    logits = (30.0 * np.tanh(logits / 30.0)).astype(np.float32)
    lmax = logits.max(axis=-1, keepdims=True)
    p = np.exp(logits - lmax)
    probs = p / p.sum(axis=-1, keepdims=True)
    idx = np.argsort(-probs, axis=-1, kind="stable")[:, :TOP_K]
    vals = np.take_along_axis(probs, idx, axis=-1)
    combine = np.zeros((T, E), dtype=np.float32)
    np.put_along_axis(combine, idx, vals, axis=-1)
    return combine


def _get_nc(C):
    key = ("nc", C)
    if key not in _CACHE:
        BLK = C // 2
        assert BLK * 2 == C and BLK % 16 == 0 and BLK <= 512, (C, BLK)
        _CACHE[key] = build_nc(C, BLK)
    return _CACHE[key]


def _tile_x(x1T, x3T):
    """two [H, C] -> [128, HC*2*C]: row p, col (c*2+j)*C+t = x{1,3}T[c*128+p, t]."""
    HH, C = x1T.shape
    both = np.stack([x1T, x3T], axis=0)                  # [2, H, C] -> j, (c p), t
    return np.ascontiguousarray(
        both.reshape(2, HC, 128, C)                      # j c p t
        .transpose(2, 1, 0, 3)                           # p c j t
        .reshape(128, HC * 2 * C)
    )


def kernel(hidden_states, gate_w, w1, w2, w3, trace=False):
    hidden_states = np.asarray(hidden_states, dtype=np.float32)
    gate_w = np.asarray(gate_w, dtype=np.float32)
    w1 = np.asarray(w1, dtype=np.float32)
    w2 = np.asarray(w2, dtype=np.float32)
    w3 = np.asarray(w3, dtype=np.float32)

    combine = _route(hidden_states, gate_w)
    tids = [np.nonzero(combine[:, e])[0] for e in range(E)]
    max_n = max(len(t) for t in tids)
    # fixed capacity; bump in 32-token steps if an input routes more
    C = max(544, -(-max_n // 32) * 32)
    nc = _get_nc(C)

    in_maps = []
    for e in range(NCORES):
        n = len(tids[e])
        xg = hidden_states[tids[e]]                       # [n, H]
        ce = combine[tids[e], e][:, None]                 # [n, 1]
        x1p = np.zeros((C, H), dtype=np.float16)
        x1p[:n] = xg
        x3p = np.zeros((C, H), dtype=np.float16)
        x3p[:n] = xg * ce

        # w13d[f*128+p, j*HC*128 + c*128 + m] = w{1,3}[e][f*128+m, c*128+p]
        w13 = np.stack(
            [w1[e].astype(np.float16), w3[e].astype(np.float16)], axis=0
        )  # [2, F, H]
        w13d = np.ascontiguousarray(
            w13.reshape(2, FC, 128, HC, 128)     # j, f, m, c, p
            .transpose(1, 4, 0, 3, 2)            # f, p, j, c, m
            .reshape(FC * 128, 2 * HC * 128)
        )
        # w2d[h*128+p, f*128+m] = w2[e].T[f*128+p, h*128+m] = w2[e][h*128+m, f*128+p]
        w2d = np.ascontiguousarray(
            w2[e].astype(np.float16)
            .reshape(HC, 128, FC, 128)           # h, m, f, p
            .transpose(0, 3, 2, 1)               # h, p, f, m
            .reshape(HC * 128, FC * 128)
        )
        in_maps.append(
            {
                "xd": _tile_x(x1p.T, x3p.T),
                "w13d": w13d,
                "w2d": w2d,
            }
        )

    res = run_bass_kernel_spmd(nc, in_maps, core_ids=list(range(NCORES)), trace=trace)
    out = np.zeros((T, H), dtype=np.float32)
    for e, r in enumerate(res.results):
        n = len(tids[e])
        out[tids[e]] += r["out"][:, :n].T.astype(np.float32)
    _CACHE["last_results"] = res
    return out


if __name__ == "__main__":
    rng = np.random.default_rng(0)
    hs = rng.standard_normal((T, H), dtype=np.float32)
    gw = (rng.standard_normal((E, H)) * 0.02).astype(np.float32)
    w1 = (rng.standard_normal((E, F, H)) * 0.02).astype(np.float32)
    w2 = (rng.standard_normal((E, H, F)) * 0.02).astype(np.float32)
    w3 = (rng.standard_normal((E, F, H)) * 0.02).astype(np.float32)
    out = kernel(hs, gw, w1, w2, w3)
    print("out", out.shape, out.dtype, np.abs(out).max())


# revision 19
# speedup vs baseline: 1.0432x; 1.0152x over previous
"""Grok1 MoE (unfused) Trainium2 Bass kernel — sparse expert-parallel.

Top-2-of-8 routing means only ~T/4 of the 2048 tokens are live per
expert.  The router (a 2048x1024x8 matmul + softmax + top-2, ~34 MFLOP)
runs on host as part of the sharding step; each of the 8 NeuronCores
owns one expert and receives only that expert's routed tokens, gathered
and padded to a fixed capacity C:

  core e computes  outT = w2 @ (gelu(w1 @ x1) * (w3 @ x3))  [H, C]

with x1 = x[tids_e].T and x3 = (combine_e * x[tids_e]).T — the combine
weight is folded into the w3 operand (u is linear in x), so no
post-scale pass is needed.  Host scatter-adds the 8 partial outputs
back to the full [T, H] result.

Device schedule notes:
  - all inputs are pre-tiled on host into the exact SBUF partition
    layout, so every DMA reads multi-KB contiguous runs per partition
    (cheap descriptors, fast queue dispatch);
  - w1/w3 are fused into one tensor (one DMA per f-chunk); x streams in
    h-chunk halves so the first matmul can issue right after the NEFF
    preamble;
  - each 128x128 weight tile stays stationary for both token blocks
    (b0/b1 interleaved into separate PSUM banks);
  - fp16 matmuls accumulate in fp32 PSUM; partial outputs evict as fp16.
"""

import numpy as np

import concourse.bass as bass
import concourse.mybir as mybir
import concourse.tile as tile
from concourse import bacc
from concourse.bass import ts
from concourse.bass_utils import run_bass_kernel_spmd

T, H, F, E = 2048, 1024, 4096, 8
NCORES = 8
TOP_K = 2
HC = H // 128   # 8 h-chunks
FC = F // 128   # 32 f-chunks

f32 = mybir.dt.float32
f16 = mybir.dt.float16

_CACHE = {}


def build_nc(C, BLK):
    """Per-expert MLP over C tokens (2 blocks of BLK)."""
    nc = bacc.Bacc(
        "TRN2",
        target_bir_lowering=False,
        debug=False,
        num_devices=NCORES,
    )

    # host-pre-tiled layouts (partition-contiguous):
    #   xd      [128, HC*2*C]    : row p, col (c*2+j)*C+t = x{1,3}.T[c*128+p, t]
    #   w13d    [FC*128, 2*HC*128]: row f*128+p, col j*HC*128+c*128+m
    #                               = w{1,3}.T[c*128+p, f*128+m]
    #   w2d     [HC*128, FC*128] : row h*128+p, col f*128+m
    #                               = w2.T[f*128+p, h*128+m]
    xd = nc.dram_tensor("xd", [128, HC * 2 * C], f16, kind="ExternalInput")
    w13d = nc.dram_tensor("w13d", [FC * 128, 2 * HC * 128], f16, kind="ExternalInput")
    w2d = nc.dram_tensor("w2d", [HC * 128, FC * 128], f16, kind="ExternalInput")
    out = nc.dram_tensor("out", [H, C], f16, kind="ExternalOutput")

    AF = mybir.ActivationFunctionType

    with tile.TileContext(nc) as tc:
        with (
            tc.tile_pool(name="singles", bufs=1) as singles,
            tc.tile_pool(name="wpool", bufs=3) as wpool,
            tc.tile_pool(name="w2pool", bufs=3) as w2pool,
            tc.tile_pool(name="evict", bufs=2) as evict,
            tc.tile_pool(name="psum_gu", bufs=1, space="PSUM") as psum_gu,
            tc.tile_pool(name="psum_o", bufs=2, space="PSUM") as psum_o,
        ):
            def load_wf(f):
                # two half-DMAs: the g-matmuls only depend on the w1 half
                w1f = wpool.tile([128, HC, 128], f16, tag="w1f", name=f"w1f_{f}")
                nc.sync.dma_start(
                    out=w1f,
                    in_=w13d.ap()[ts(f, 128), 0 : HC * 128].rearrange(
                        "p (c m) -> p c m", c=HC
                    ),
                )
                w3f = wpool.tile([128, HC, 128], f16, tag="w3f", name=f"w3f_{f}")
                nc.sync.dma_start(
                    out=w3f,
                    in_=w13d.ap()[ts(f, 128), HC * 128 : 2 * HC * 128].rearrange(
                        "p (c m) -> p c m", c=HC
                    ),
                )
                return w1f, w3f

            # queue order: f=0 weights, then x in h-chunk quarters (x1/x3
            # interleaved) so the first g-matmuls start right after the
            # NEFF preamble
            wf_next = load_wf(0)
            x_sb = singles.tile([128, HC, 2, C], f16)
            for lo in range(0, HC, 2):
                nc.sync.dma_start(
                    out=x_sb[:, lo : lo + 2, :, :],
                    in_=xd.ap()[:, lo * 2 * C : (lo + 2) * 2 * C].rearrange(
                        "p (c j t) -> p c j t", c=2, j=2
                    ),
                )
            x1_sb = x_sb[:, :, 0, :]
            x3_sb = x_sb[:, :, 1, :]
            gus = singles.tile([128, FC, C], f16)

            # ---- PE warm-up: dummy matmuls while the first DMAs are in
            # flight, so the HAM clock gate releases (1.2 -> 2.4 GHz)
            # before the real matmul stream begins ----
            warm_sb = singles.tile([128, 128], f16)
            nc.vector.memset(warm_sb, 0.0)
            warm_ps = psum_o.tile([128, 128], f32, tag="o0", name="warm_ps")
            for i in range(56):
                nc.tensor.matmul(warm_ps, lhsT=warm_sb, rhs=warm_sb)

            # ---- phase W: g/u matmuls + gelu + mul -> gus (fp16) ----
            for f in range(FC):
                w1f, w3f = wf_next
                wf_next = load_wf(f + 1) if f + 1 < FC else None
                gb = [
                    psum_gu.tile([128, BLK], f32, tag=f"g{b}", name=f"g{b}_{f}")
                    for b in range(2)
                ]
                ub = [
                    psum_gu.tile([128, BLK], f32, tag=f"u{b}", name=f"u{b}_{f}")
                    for b in range(2)
                ]
                for h in range(HC):
                    for b in range(2):
                        nc.tensor.matmul(
                            gb[b],
                            lhsT=w1f[:, h, :],
                            rhs=x1_sb[:, h, ts(b, BLK)],
                            start=(h == 0),
                            stop=(h == HC - 1),
                        )
                gs = []
                for b in range(2):
                    g_sb = evict.tile([128, BLK], f32, tag=f"gs{b}", name=f"gs{b}_{f}")
                    nc.scalar.activation(g_sb, gb[b], AF.Gelu)
                    gs.append(g_sb)
                for h in range(HC):
                    for b in range(2):
                        nc.tensor.matmul(
                            ub[b],
                            lhsT=w3f[:, h, :],
                            rhs=x3_sb[:, h, ts(b, BLK)],
                            start=(h == 0),
                            stop=(h == HC - 1),
                        )
                for b in range(2):
                    nc.vector.tensor_mul(gus[:, f, ts(b, BLK)], gs[b], ub[b])

            # ---- phase M2: out[h,:] = w2 @ gus ----
            for h in range(HC):
                halves = []
                for hv in range(2):
                    w2h = w2pool.tile(
                        [128, FC // 2, 128], f16, tag=f"w2h{hv}", name=f"w2h{hv}_{h}"
                    )
                    nc.sync.dma_start(
                        out=w2h,
                        in_=w2d.ap()[
                            ts(h, 128), hv * (FC // 2) * 128 : (hv + 1) * (FC // 2) * 128
                        ].rearrange("p (c m) -> p c m", c=FC // 2),
                    )
                    halves.append(w2h)
                ob = [
                    psum_o.tile([128, BLK], f32, tag=f"o{b}", name=f"o{b}_{h}")
                    for b in range(2)
                ]
                for f in range(FC):
                    for b in range(2):
                        nc.tensor.matmul(
                            ob[b],
                            lhsT=halves[f // (FC // 2)][:, f % (FC // 2), :],
                            rhs=gus[:, f, ts(b, BLK)],
                            start=(f == 0),
                            stop=(f == FC - 1),
                        )
                for b in range(2):
                    o_sb = evict.tile(
                        [128, BLK], f16, tag=f"osb{b}", name=f"osb{b}_{h}"
                    )
                    # alternate engines so the two evictions run concurrently
                    if b == 0:
                        nc.scalar.copy(o_sb, ob[b])
                    else:
                        nc.vector.tensor_copy(o_sb, ob[b])
                    nc.sync.dma_start(out=out.ap()[ts(h, 128), ts(b, BLK)], in_=o_sb)
    nc.finalize()
    return nc


def _route(hidden_states, gate_w):
    """fp32 router identical to the reference: softcapped logits ->
    softmax -> top-2 -> combine weights [T, E]."""
    logits = hidden_states @ gate_w.T
    logits = (30.0 * np.tanh(logits / 30.0)).astype(np.float32)
    lmax = logits.max(axis=-1, keepdims=True)
    p = np.exp(logits - lmax)
    probs = p / p.sum(axis=-1, keepdims=True)
    idx = np.argsort(-probs, axis=-1, kind="stable")[:, :TOP_K]
    vals = np.take_along_axis(probs, idx, axis=-1)
    combine = np.zeros((T, E), dtype=np.float32)
    np.put_along_axis(combine, idx, vals, axis=-1)
    return combine


def _get_nc(C):
    key = ("nc", C)
    if key not in _CACHE:
        BLK = C // 2
        assert BLK * 2 == C and BLK % 16 == 0 and BLK <= 512, (C, BLK)
        _CACHE[key] = build_nc(C, BLK)
    return _CACHE[key]


def _tile_x(x1T, x3T):
    """two [H, C] -> [128, HC*2*C]: row p, col (c*2+j)*C+t = x{1,3}T[c*128+p, t]."""
    HH, C = x1T.shape
    both = np.stack([x1T, x3T], axis=0)                  # [2, H, C] -> j, (c p), t
    return np.ascontiguousarray(
        both.reshape(2, HC, 128, C)                      # j c p t
        .transpose(2, 1, 0, 3)                           # p c j t
        .reshape(128, HC * 2 * C)
    )


def kernel(hidden_states, gate_w, w1, w2, w3, trace=False):
    hidden_states = np.asarray(hidden_states, dtype=np.float32)
    gate_w = np.asarray(gate_w, dtype=np.float32)
    w1 = np.asarray(w1, dtype=np.float32)
    w2 = np.asarray(w2, dtype=np.float32)
    w3 = np.asarray(w3, dtype=np.float32)

    combine = _route(hidden_states, gate_w)
    tids = [np.nonzero(combine[:, e])[0] for e in range(E)]
    max_n = max(len(t) for t in tids)
    # fixed capacity; bump in 32-token steps if an input routes more
    C = max(544, -(-max_n // 32) * 32)
    nc = _get_nc(C)

    in_maps = []
    for e in range(NCORES):
        n = len(tids[e])
        xg = hidden_states[tids[e]]                       # [n, H]
        ce = combine[tids[e], e][:, None]                 # [n, 1]
        x1p = np.zeros((C, H), dtype=np.float16)
        x1p[:n] = xg
        x3p = np.zeros((C, H), dtype=np.float16)
        x3p[:n] = xg * ce

        # w13d[f*128+p, j*HC*128 + c*128 + m] = w{1,3}[e][f*128+m, c*128+p]
        w13 = np.stack(
            [w1[e].astype(np.float16), w3[e].astype(np.float16)], axis=0
        )  # [2, F, H]
        w13d = np.ascontiguousarray(
            w13.reshape(2, FC, 128, HC, 128)     # j, f, m, c, p
            .transpose(1, 4, 0, 3, 2)            # f, p, j, c, m
            .reshape(FC * 128, 2 * HC * 128)
        )
        # w2d[h*128+p, f*128+m] = w2[e].T[f*128+p, h*128+m] = w2[e][h*128+m, f*128+p]
        w2d = np.ascontiguousarray(
            w2[e].astype(np.float16)
            .reshape(HC, 128, FC, 128)           # h, m, f, p
            .transpose(0, 3, 2, 1)               # h, p, f, m
            .reshape(HC * 128, FC * 128)
        )
        in_maps.append(
            {
                "xd": _tile_x(x1p.T, x3p.T),
                "w13d": w13d,
                "w2d": w2d,
            }
        )

    res = run_bass_kernel_spmd(nc, in_maps, core_ids=list(range(NCORES)), trace=trace)
    out = np.zeros((T, H), dtype=np.float32)
    for e, r in enumerate(res.results):
        n = len(tids[e])
        out[tids[e]] += r["out"][:, :n].T.astype(np.float32)
    _CACHE["last_results"] = res
    return out


if __name__ == "__main__":
    rng = np.random.default_rng(0)
    hs = rng.standard_normal((T, H), dtype=np.float32)
    gw = (rng.standard_normal((E, H)) * 0.02).astype(np.float32)
    w1 = (rng.standard_normal((E, F, H)) * 0.02).astype(np.float32)
    w2 = (rng.standard_normal((E, H, F)) * 0.02).astype(np.float32)
    w3 = (rng.standard_normal((E, F, H)) * 0.02).astype(np.float32)
    out = kernel(hs, gw, w1, w2, w3)
    print("out", out.shape, out.dtype, np.abs(out).max())
